# revision 11
# baseline (speedup 1.0000x reference)
"""Trainium2 Bass kernel for Group_EB_MLP (embedding-bag mean + tiny MLP).

Model (per reference):
    eb_out  = segment_mean(emb_weight[eb_input], eb_offset)     # [B, 3]
    mlp_out = mlp_input @ W0.T+b0 @ W1.T+b1 @ W2.T+b2           # [B, 3] (pure affine)
    out     = concat([eb_out, eb_out, eb_out, mlp_out], axis=1) # [B, 12]

Sharding: data-parallel over bags across 8 NeuronCores (2048 bags/core);
the 10M x 3 embedding table is replicated in each core's HBM.

The only heavy device work is the gather: 102400 random 12-byte rows per
core via SWDGE indirect DMA (~0.13 ns/descriptor when the generation
stream never stalls). v3 keeps that stream saturated and strips
everything else off the critical path:
  - indices arrive in per-chunk [128, gpc*slots] DMAs; 4 indirect-gather
    chunks of 4 groups run back-to-back,
  - the per-bag mean: the 1/count scale is folded into the table on the
    host when counts are uniform (same algebraic folding as the MLP
    weights), so VectorE does ONE strided reduce per group, writing
    straight into the output tile,
  - MLP: out.T = weff.T @ xt computed as 4 matmuls [3, 512] with the
    tiny weff stationary (instead of 16 [14x128] reloads), stored as
    [3, 2048]; the host interleaves columns and replicates the three
    identical eb column blocks (pure data movement, no arithmetic),
  - one [128, groups*3] eb store at the end.

The three linear layers have no activations between them, so they fold
into a single affine map (Weff, beff) on the host.
"""

import numpy as np

import concourse.bass as bass
import concourse.tile as tile
from concourse import bacc, mybir
from concourse.bass_utils import run_bass_kernel_spmd

B = 16384
L = 50
N = B * L
V = 10_000_000
D = 3
K = 13
NCORES = 8
GROUPS_PER_CHUNK = 4
MM_COLS = 512  # one PSUM bank of fp32

_PROG_CACHE = {}


def _chunk_groups(groups):
    """Uneven gather chunks: small first chunk so the SWDGE descriptor
    stream starts as early as possible; small-ish last chunk for the tail."""
    if groups == 16:
        return [2, 6, 6, 2]
    if groups % GROUPS_PER_CHUNK == 0:
        return [GROUPS_PER_CHUNK] * (groups // GROUPS_PER_CHUNK)
    return [1] * groups


def _build_program(v_rows, d, k, groups, slots, uniform):
    """Per-core SPMD program: groups*128 bags, `slots` padded indices/bag."""
    nc = bacc.Bacc("TRN2", debug=False)
    f32 = mybir.dt.float32
    i32 = mybir.dt.int32
    b_loc = groups * 128
    cg = _chunk_groups(groups)
    chunks = len(cg)
    offs = [sum(cg[:i]) for i in range(chunks)]
    max_gpc = max(cg)
    mm_chunks = max(b_loc // MM_COLS, 1)

    table = nc.declare_dram_parameter("table", [v_rows, d], f32, isOutput=False)
    idx = nc.declare_dram_parameter("idx", [128, groups * slots], i32, isOutput=False)
    xt = nc.declare_dram_parameter("xt", [k + 1, b_loc], f32, isOutput=False)
    weff = nc.declare_dram_parameter("weff", [k + 1, d], f32, isOutput=False)
    out_eb = nc.declare_dram_parameter("out_eb", [128, groups * d], f32, isOutput=True)
    out_mlp = nc.declare_dram_parameter("out_mlp", [d, b_loc], f32, isOutput=True)
    if not uniform:
        invc = nc.declare_dram_parameter("invc", [128, groups], f32, isOutput=False)

    with tile.TileContext(nc) as tc:
        with (
            tc.tile_pool(name="const", bufs=1) as cpool,
            tc.tile_pool(name="work", bufs=3) as wpool,
            tc.tile_pool(name="psum", bufs=4, space="PSUM") as ppool,
        ):
            # per-chunk index tiles; chunk 0 loads first so gather 0 can
            # issue as early as possible
            idx_sb = [
                cpool.tile([128, cg[c] * slots], i32, name=f"idx_sb{c}")
                for c in range(chunks)
            ]
            nc.sync.dma_start(
                out=idx_sb[0][:],
                in_=idx[:, offs[0] * slots : (offs[0] + cg[0]) * slots],
            )

            eb_sb = cpool.tile([128, groups * d], f32)
            for c in range(chunks):
                gpc, goff = cg[c], offs[c]
                if c + 1 < chunks:
                    nc.sync.dma_start(
                        out=idx_sb[c + 1][:],
                        in_=idx[
                            :, offs[c + 1] * slots : (offs[c + 1] + cg[c + 1]) * slots
                        ],
                    )
                gat = wpool.tile([128, max_gpc * slots * d], f32, tag="gat")
                nc.gpsimd.indirect_dma_start(
                    out=gat[:, : gpc * slots * d],
                    out_offset=None,
                    in_=table[:],
                    in_offset=bass.IndirectOffsetOnAxis(ap=idx_sb[c][:], axis=0),
                )

                if c == 0:
                    # independent MLP chain rides under the gather stream.
                    # Everything stays off the Scalar engine so it is never
                    # used at all (saves its ACT-table preamble load and
                    # start/end barrier participation).
                    weff_sb = cpool.tile([k + 1, d], f32)
                    nc.sync.dma_start(out=weff_sb[:], in_=weff[:])
                    xt_sb = cpool.tile([k + 1, b_loc], f32)
                    nc.sync.dma_start(out=xt_sb[:], in_=xt[:])
                    if not uniform:
                        invc_sb = cpool.tile([128, groups], f32)
                        nc.sync.dma_start(out=invc_sb[:], in_=invc[:])
                    mlp_sb = cpool.tile([d, b_loc], f32)
                    for m in range(mm_chunks):
                        ps = ppool.tile([d, MM_COLS], f32, space="PSUM")
                        nc.tensor.matmul(
                            out=ps[:],
                            lhsT=weff_sb[:],
                            rhs=xt_sb[:, m * MM_COLS : (m + 1) * MM_COLS],
                            start=True,
                            stop=True,
                        )
                        nc.vector.tensor_copy(
                            out=mlp_sb[:, m * MM_COLS : (m + 1) * MM_COLS], in_=ps[:]
                        )
                    nc.sync.dma_start(out=out_mlp[:], in_=mlp_sb[:])

                for j in range(gpc):
                    g = goff + j
                    if uniform:
                        nc.vector.tensor_reduce(
                            out=eb_sb[:, g * d : (g + 1) * d],
                            in_=gat[:, j * slots * d : (j + 1) * slots * d].rearrange(
                                "p (f e) -> p e f", e=d
                            ),
                            axis=mybir.AxisListType.X,
                            op=mybir.AluOpType.add,
                        )
                    else:
                        sums = wpool.tile([128, d], f32, tag="sums")
                        nc.vector.tensor_reduce(
                            out=sums[:],
                            in_=gat[:, j * slots * d : (j + 1) * slots * d].rearrange(
                                "p (f e) -> p e f", e=d
                            ),
                            axis=mybir.AxisListType.X,
                            op=mybir.AluOpType.add,
                        )
                        nc.vector.tensor_tensor(
                            out=eb_sb[:, g * d : (g + 1) * d],
                            in0=sums[:],
                            in1=invc_sb[:, g : g + 1].to_broadcast([128, d]),
                            op=mybir.AluOpType.mult,
                        )

                # stream this chunk's bag means out while later gathers run
                nc.sync.dma_start(
                    out=out_eb[:, goff * d : (goff + gpc) * d],
                    in_=eb_sb[:, goff * d : (goff + gpc) * d],
                )

    nc.compile()
    return nc


def _get_program(v_rows, d, k, groups, slots, uniform):
    key = (v_rows, d, k, groups, slots, uniform)
    if key not in _PROG_CACHE:
        _PROG_CACHE[key] = _build_program(v_rows, d, k, groups, slots, uniform)
    return _PROG_CACHE[key]


def _prepare(eb_input, eb_offset, mlp_input, emb_weight, w0, b0, w1, b1, w2, b2):
    """Shard/pack the full inputs into per-core input maps."""
    eb_input = np.ascontiguousarray(np.asarray(eb_input, dtype=np.int32))
    eb_offset = np.asarray(eb_offset).astype(np.int64)
    mlp_input = np.asarray(mlp_input, dtype=np.float32)
    emb_weight = np.ascontiguousarray(np.asarray(emb_weight, dtype=np.float32))

    n = int(eb_input.shape[0])
    b = int(eb_offset.shape[0])
    v, d = emb_weight.shape
    k = int(mlp_input.shape[1])
    assert b % (NCORES * 128) == 0, f"B={b} must divide across {NCORES} cores x 128"
    b_loc = b // NCORES
    groups = b_loc // 128

    counts = np.diff(np.append(eb_offset, n))
    uniform = int(eb_offset[0]) == 0 and bool(np.all(counts == counts[0]))
    if uniform:
        slots = int(counts[0])
        idx_mat = eb_input.reshape(b, slots)
        table = np.concatenate(
            [emb_weight * np.float32(1.0 / counts[0]), np.zeros((1, d), np.float32)],
            axis=0,
        )
        inv = None
    else:
        # general sorted-offset path: pad each bag to `slots` with index v
        # (an appended all-zeros table row), so padding contributes 0 to sums
        slots = max(int(counts.max()), 1)
        idx_mat = np.full((b, slots), v, dtype=np.int32)
        ar = np.arange(n, dtype=np.int64)
        bag_ids = np.searchsorted(eb_offset, ar, side="right") - 1
        pos = ar - eb_offset[bag_ids]
        idx_mat[bag_ids, pos] = eb_input
        table = np.concatenate([emb_weight, np.zeros((1, d), np.float32)], axis=0)
        with np.errstate(divide="ignore"):
            inv = (1.0 / counts.astype(np.float64)).astype(np.float32)

    # fold the activation-free 3-layer MLP into one affine map
    w0d, w1d, w2d = (np.asarray(w, dtype=np.float64) for w in (w0, w1, w2))
    b0d, b1d, b2d = (np.asarray(x, dtype=np.float64) for x in (b0, b1, b2))
    w_eff = (w2d @ w1d @ w0d).T  # [K, 3]
    b_eff = b2d + b1d @ w2d.T + b0d @ (w2d @ w1d).T  # [3]
    weff_aug = np.concatenate([w_eff, b_eff[None, :]], axis=0).astype(np.float32)

    xt_full = np.concatenate(
        [mlp_input.T, np.ones((1, b), np.float32)], axis=0
    ).astype(np.float32)  # [K+1, B]

    in_maps = []
    for c in range(NCORES):
        sl = slice(c * b_loc, (c + 1) * b_loc)
        # bag (g, p) -> partition p, slot block g: [128, groups*slots]
        idx_c = (
            idx_mat[sl]
            .reshape(groups, 128, slots)
            .transpose(1, 0, 2)
            .reshape(128, groups * slots)
        )
        im = {
            "table": table,
            "idx": np.ascontiguousarray(idx_c),
            "xt": np.ascontiguousarray(xt_full[:, sl]),
            "weff": weff_aug,
        }
        if not uniform:
            im["invc"] = np.ascontiguousarray(inv[sl].reshape(groups, 128).T)
        in_maps.append(im)
    dims = dict(
        v_rows=v + 1, d=d, k=k, groups=groups, slots=slots, b_loc=b_loc,
        uniform=uniform,
    )
    return in_maps, dims


def _run(in_maps, dims, trace=False):
    nc = _get_program(
        dims["v_rows"], dims["d"], dims["k"], dims["groups"], dims["slots"],
        dims["uniform"],
    )
    res = run_bass_kernel_spmd(nc, in_maps, list(range(NCORES)), trace=trace)
    groups, d, b_loc = dims["groups"], dims["d"], dims["b_loc"]
    out = np.empty((NCORES * b_loc, 4 * d), dtype=np.float32)
    for c in range(NCORES):
        r = res.results[c]
        # eb: [128, groups*d] with bag (g, p) at [p, g*d:(g+1)*d]
        eb = (
            r["out_eb"].reshape(128, groups, d).transpose(1, 0, 2).reshape(b_loc, d)
        )
        mlp = r["out_mlp"].reshape(d, b_loc).T  # [b_loc, d]
        blk = out[c * b_loc : (c + 1) * b_loc]
        blk[:, 0 * d : 1 * d] = eb
        blk[:, 1 * d : 2 * d] = eb
        blk[:, 2 * d : 3 * d] = eb
        blk[:, 3 * d : 4 * d] = mlp
    return out, res


def kernel(eb_input, eb_offset, mlp_input, emb_weight, w0, b0, w1, b1, w2, b2):
    in_maps, dims = _prepare(
        eb_input, eb_offset, mlp_input, emb_weight, w0, b0, w1, b1, w2, b2
    )
    out, _ = _run(in_maps, dims, trace=False)
    return out


def kernel_profiled(**inputs):
    """Like kernel(), but also returns the BassKernelResults with HW timing."""
    in_maps, dims = _prepare(**inputs)
    return _run(in_maps, dims, trace=True)


# revision 13
# speedup vs baseline: 1.1434x; 1.1434x over previous
"""Trainium2 Bass kernel for Group_EB_MLP (embedding-bag mean + tiny MLP).

Model (per reference):
    eb_out  = segment_mean(emb_weight[eb_input], eb_offset)     # [B, 3]
    mlp_out = mlp_input @ W0.T+b0 @ W1.T+b1 @ W2.T+b2           # [B, 3] (pure affine)
    out     = concat([eb_out, eb_out, eb_out, mlp_out], axis=1) # [B, 12]

Sharding: data-parallel over bags across 8 NeuronCores (2048 bags/core);
the 10M x 3 embedding table is replicated in each core's HBM.

The only heavy device work is the gather: 102400 random 12-byte rows per
core via SWDGE indirect DMA (~0.13 ns/descriptor when the generation
stream never stalls). v3 keeps that stream saturated and strips
everything else off the critical path:
  - indices arrive in per-chunk [128, gpc*slots] DMAs; 4 indirect-gather
    chunks of 4 groups run back-to-back,
  - the per-bag mean: the 1/count scale is folded into the table on the
    host when counts are uniform (same algebraic folding as the MLP
    weights), so VectorE does ONE strided reduce per group, writing
    straight into the output tile,
  - MLP: out.T = weff.T @ xt computed as 4 matmuls [3, 512] with the
    tiny weff stationary (instead of 16 [14x128] reloads), stored as
    [3, 2048]; the host interleaves columns and replicates the three
    identical eb column blocks (pure data movement, no arithmetic),
  - one [128, groups*3] eb store at the end.

The three linear layers have no activations between them, so they fold
into a single affine map (Weff, beff) on the host.
"""

import numpy as np

import concourse.bass as bass
import concourse.tile as tile
from concourse import bacc, mybir
from concourse.bass_utils import run_bass_kernel_spmd

B = 16384
L = 50
N = B * L
V = 10_000_000
D = 3
K = 13
NCORES = 8
GROUPS_PER_CHUNK = 4
MM_COLS = 512  # one PSUM bank of fp32

_PROG_CACHE = {}


def _chunk_groups(groups):
    """Uneven gather chunks: small first chunk so the SWDGE descriptor
    stream starts as early as possible; small-ish last chunk for the tail."""
    if groups == 16:
        return [2, 5, 5, 4]
    if groups % GROUPS_PER_CHUNK == 0:
        return [GROUPS_PER_CHUNK] * (groups // GROUPS_PER_CHUNK)
    return [1] * groups


def _build_program(v_rows, d, k, groups, slots, uniform):
    """Per-core SPMD program: groups*128 bags, `slots` padded indices/bag."""
    nc = bacc.Bacc("TRN2", debug=False)
    f32 = mybir.dt.float32
    i32 = mybir.dt.int32
    b_loc = groups * 128
    cg = _chunk_groups(groups)
    chunks = len(cg)
    offs = [sum(cg[:i]) for i in range(chunks)]
    max_gpc = max(cg)
    mm_chunks = max(b_loc // MM_COLS, 1)

    table = nc.declare_dram_parameter("table", [v_rows, d], f32, isOutput=False)
    idx = nc.declare_dram_parameter("idx", [128, groups * slots], i32, isOutput=False)
    xt = nc.declare_dram_parameter("xt", [k + 1, b_loc], f32, isOutput=False)
    weff = nc.declare_dram_parameter("weff", [k + 1, d], f32, isOutput=False)
    out_eb = nc.declare_dram_parameter("out_eb", [128, groups * d], f32, isOutput=True)
    out_mlp = nc.declare_dram_parameter("out_mlp", [d, b_loc], f32, isOutput=True)
    if not uniform:
        invc = nc.declare_dram_parameter("invc", [128, groups], f32, isOutput=False)

    with tile.TileContext(nc) as tc:
        with (
            tc.tile_pool(name="const", bufs=1) as cpool,
            tc.tile_pool(name="work", bufs=3) as wpool,
            tc.tile_pool(name="psum", bufs=4, space="PSUM") as ppool,
        ):
            # per-chunk index tiles; chunk 0 loads first so gather 0 can
            # issue as early as possible
            idx_sb = [
                cpool.tile([128, cg[c] * slots], i32, name=f"idx_sb{c}")
                for c in range(chunks)
            ]
            nc.sync.dma_start(
                out=idx_sb[0][:],
                in_=idx[:, offs[0] * slots : (offs[0] + cg[0]) * slots],
            )

            eb_sb = cpool.tile([128, groups * d], f32)
            for c in range(chunks):
                gpc, goff = cg[c], offs[c]
                if c + 1 < chunks:
                    nc.sync.dma_start(
                        out=idx_sb[c + 1][:],
                        in_=idx[
                            :, offs[c + 1] * slots : (offs[c + 1] + cg[c + 1]) * slots
                        ],
                    )
                gat = wpool.tile([128, max_gpc * slots * d], f32, tag="gat")
                nc.gpsimd.indirect_dma_start(
                    out=gat[:, : gpc * slots * d],
                    out_offset=None,
                    in_=table[:],
                    in_offset=bass.IndirectOffsetOnAxis(ap=idx_sb[c][:], axis=0),
                )

                if c == 0:
                    # independent MLP chain rides under the gather stream;
                    # its loads/copies/store live on the Scalar engine's
                    # HWDGE queue so they never delay the Sync-engine idx
                    # loads that feed the SWDGE gather stream.
                    weff_sb = cpool.tile([k + 1, d], f32)
                    nc.scalar.dma_start(out=weff_sb[:], in_=weff[:])
                    xt_sb = cpool.tile([k + 1, b_loc], f32)
                    nc.scalar.dma_start(out=xt_sb[:], in_=xt[:])
                    if not uniform:
                        invc_sb = cpool.tile([128, groups], f32)
                        nc.scalar.dma_start(out=invc_sb[:], in_=invc[:])
                    mlp_sb = cpool.tile([d, b_loc], f32)
                    for m in range(mm_chunks):
                        ps = ppool.tile([d, MM_COLS], f32, space="PSUM")
                        nc.tensor.matmul(
                            out=ps[:],
                            lhsT=weff_sb[:],
                            rhs=xt_sb[:, m * MM_COLS : (m + 1) * MM_COLS],
                            start=True,
                            stop=True,
                        )
                        nc.scalar.copy(
                            out=mlp_sb[:, m * MM_COLS : (m + 1) * MM_COLS], in_=ps[:]
                        )
                    nc.scalar.dma_start(out=out_mlp[:], in_=mlp_sb[:])

                for j in range(gpc):
                    g = goff + j
                    if uniform:
                        nc.vector.tensor_reduce(
                            out=eb_sb[:, g * d : (g + 1) * d],
                            in_=gat[:, j * slots * d : (j + 1) * slots * d].rearrange(
                                "p (f e) -> p e f", e=d
                            ),
                            axis=mybir.AxisListType.X,
                            op=mybir.AluOpType.add,
                        )
                    else:
                        sums = wpool.tile([128, d], f32, tag="sums")
                        nc.vector.tensor_reduce(
                            out=sums[:],
                            in_=gat[:, j * slots * d : (j + 1) * slots * d].rearrange(
                                "p (f e) -> p e f", e=d
                            ),
                            axis=mybir.AxisListType.X,
                            op=mybir.AluOpType.add,
                        )
                        nc.vector.tensor_tensor(
                            out=eb_sb[:, g * d : (g + 1) * d],
                            in0=sums[:],
                            in1=invc_sb[:, g : g + 1].to_broadcast([128, d]),
                            op=mybir.AluOpType.mult,
                        )

                # stream this chunk's bag means out while later gathers run
                nc.sync.dma_start(
                    out=out_eb[:, goff * d : (goff + gpc) * d],
                    in_=eb_sb[:, goff * d : (goff + gpc) * d],
                )

    nc.compile()
    return nc


def _get_program(v_rows, d, k, groups, slots, uniform):
    key = (v_rows, d, k, groups, slots, uniform)
    if key not in _PROG_CACHE:
        _PROG_CACHE[key] = _build_program(v_rows, d, k, groups, slots, uniform)
    return _PROG_CACHE[key]


def _prepare(eb_input, eb_offset, mlp_input, emb_weight, w0, b0, w1, b1, w2, b2):
    """Shard/pack the full inputs into per-core input maps."""
    eb_input = np.ascontiguousarray(np.asarray(eb_input, dtype=np.int32))
    eb_offset = np.asarray(eb_offset).astype(np.int64)
    mlp_input = np.asarray(mlp_input, dtype=np.float32)
    emb_weight = np.ascontiguousarray(np.asarray(emb_weight, dtype=np.float32))

    n = int(eb_input.shape[0])
    b = int(eb_offset.shape[0])
    v, d = emb_weight.shape
    k = int(mlp_input.shape[1])
    assert b % (NCORES * 128) == 0, f"B={b} must divide across {NCORES} cores x 128"
    b_loc = b // NCORES
    groups = b_loc // 128

    counts = np.diff(np.append(eb_offset, n))
    uniform = int(eb_offset[0]) == 0 and bool(np.all(counts == counts[0]))
    if uniform:
        slots = int(counts[0])
        idx_mat = eb_input.reshape(b, slots)
        table = np.concatenate(
            [emb_weight * np.float32(1.0 / counts[0]), np.zeros((1, d), np.float32)],
            axis=0,
        )
        inv = None
    else:
        # general sorted-offset path: pad each bag to `slots` with index v
        # (an appended all-zeros table row), so padding contributes 0 to sums
        slots = max(int(counts.max()), 1)
        idx_mat = np.full((b, slots), v, dtype=np.int32)
        ar = np.arange(n, dtype=np.int64)
        bag_ids = np.searchsorted(eb_offset, ar, side="right") - 1
        pos = ar - eb_offset[bag_ids]
        idx_mat[bag_ids, pos] = eb_input
        table = np.concatenate([emb_weight, np.zeros((1, d), np.float32)], axis=0)
        with np.errstate(divide="ignore"):
            inv = (1.0 / counts.astype(np.float64)).astype(np.float32)

    # fold the activation-free 3-layer MLP into one affine map
    w0d, w1d, w2d = (np.asarray(w, dtype=np.float64) for w in (w0, w1, w2))
    b0d, b1d, b2d = (np.asarray(x, dtype=np.float64) for x in (b0, b1, b2))
    w_eff = (w2d @ w1d @ w0d).T  # [K, 3]
    b_eff = b2d + b1d @ w2d.T + b0d @ (w2d @ w1d).T  # [3]
    weff_aug = np.concatenate([w_eff, b_eff[None, :]], axis=0).astype(np.float32)

    xt_full = np.concatenate(
        [mlp_input.T, np.ones((1, b), np.float32)], axis=0
    ).astype(np.float32)  # [K+1, B]

    in_maps = []
    for c in range(NCORES):
        sl = slice(c * b_loc, (c + 1) * b_loc)
        # bag (g, p) -> partition p, slot block g: [128, groups*slots]
        idx_c = (
            idx_mat[sl]
            .reshape(groups, 128, slots)
            .transpose(1, 0, 2)
            .reshape(128, groups * slots)
        )
        im = {
            "table": table,
            "idx": np.ascontiguousarray(idx_c),
            "xt": np.ascontiguousarray(xt_full[:, sl]),
            "weff": weff_aug,
        }
        if not uniform:
            im["invc"] = np.ascontiguousarray(inv[sl].reshape(groups, 128).T)
        in_maps.append(im)
    dims = dict(
        v_rows=v + 1, d=d, k=k, groups=groups, slots=slots, b_loc=b_loc,
        uniform=uniform,
    )
    return in_maps, dims


def _run(in_maps, dims, trace=False):
    nc = _get_program(
        dims["v_rows"], dims["d"], dims["k"], dims["groups"], dims["slots"],
        dims["uniform"],
    )
    res = run_bass_kernel_spmd(nc, in_maps, list(range(NCORES)), trace=trace)
    groups, d, b_loc = dims["groups"], dims["d"], dims["b_loc"]
    out = np.empty((NCORES * b_loc, 4 * d), dtype=np.float32)
    for c in range(NCORES):
        r = res.results[c]
        # eb: [128, groups*d] with bag (g, p) at [p, g*d:(g+1)*d]
        eb = (
            r["out_eb"].reshape(128, groups, d).transpose(1, 0, 2).reshape(b_loc, d)
        )
        mlp = r["out_mlp"].reshape(d, b_loc).T  # [b_loc, d]
        blk = out[c * b_loc : (c + 1) * b_loc]
        blk[:, 0 * d : 1 * d] = eb
        blk[:, 1 * d : 2 * d] = eb
        blk[:, 2 * d : 3 * d] = eb
        blk[:, 3 * d : 4 * d] = mlp
    return out, res


def kernel(eb_input, eb_offset, mlp_input, emb_weight, w0, b0, w1, b1, w2, b2):
    in_maps, dims = _prepare(
        eb_input, eb_offset, mlp_input, emb_weight, w0, b0, w1, b1, w2, b2
    )
    out, _ = _run(in_maps, dims, trace=False)
    return out


def kernel_profiled(**inputs):
    """Like kernel(), but also returns the BassKernelResults with HW timing."""
    in_maps, dims = _prepare(**inputs)
    return _run(in_maps, dims, trace=True)
